# revision 12
# baseline (speedup 1.0000x reference)
"""Trainium2 Bass kernel for the ConstraintLoss problem (8-core SPMD).

Contract: kernel(**inputs) takes the FULL unsharded inputs (numpy or jax
arrays, keyed as in setup_inputs()) and returns the full output — the
8-tuple of scalar losses stacked into a float32 array of shape (8,):
  [L_total, L_recon, L_rule, L_attn, L_attn_gat, L_attn_rule, L_reg,
   num_violations]

Sharding strategy (host side = structure prep + shard/unshard only; the
floating-point reduction math runs on the 8 NeuronCores):
  * Cars (180000) are sharded by ordinal range across the 8 cores
    (22528 rows/core, padded); score vectors follow the same row split.
  * The edge-wise segment-max over source-node segments is turned into a
    dense per-car reduction: the host bins each car's rule-edge payloads
    (payload = 1 - alpha, fp8; empty slot = 2.0) into a [rows, K=6]
    table; each core row-MIN-reduces its shard on the vector engine —
    the distributed segment-max from the sharding hint with the node
    space sharded so no cross-core combine is needed. Cars with more
    than K rule edges get the min of the extras folded into the last
    slot on the host (exact).
  * param0/param1 are shipped as fp8 (e4m3) and sharded by rows (512
    rows of each per core); each core computes its partial sum of
    squares split across THREE engines: ACT (Square+accum), DVE (STT
    square+accum), and PE — the tensor engine computes 128-col
    self-products C_k^T C_k accumulated into one PSUM tile; its
    diagonal (per-column sums of squares) is extracted once per
    iteration with an STT against a host-shipped identity matrix.
    fp8 quantization biases L_reg by ~1e-3 relative — far inside the
    2e-2 gate.
  * Each engine accumulates its scalar partials into its OWN [128, C]
    tile (single writer; cross-engine shared accum tiles race on HW)
    and each tile is DMA'd out directly; the host adds partitions and
    cores in float64 and applies the final scalar formula (the
    "all-reduce the scalar losses" step).

DMA: ~4.5 MB per core per iteration as FOUR transfers — one contiguous
column-range of the param blob per queue (sync/scalar HWDGE, gpsimd
SWDGE) plus one merged smalls tensor (tab fp8 + scores f16, bitcast
views on SBUF) — sized so the three queues finish together; the
per-core SDMA aggregate (~320-420 GB/s) is the binding resource.
"""

import numpy as np
from contextlib import ExitStack

import ml_dtypes
import concourse.bacc as bacc
import concourse.mybir as mybir
import concourse.tile as tile
from concourse.bass_utils import run_bass_kernel_spmd

F32 = mybir.dt.float32
F16 = mybir.dt.float16
BF16 = mybir.dt.bfloat16
FP8 = mybir.dt.float8e4
U8 = mybir.dt.uint8
ALU = mybir.AluOpType
ACTF = mybir.ActivationFunctionType

# Problem constants (hardcoded per the task contract).
N_CAR = 180000
N = 200000
NCORES = 8

G = 176                    # row groups per partition
RPC = 128 * G              # 22528 rows (car ordinals) per core
ROWS = RPC * NCORES        # 180224 padded rows
NPAD = ROWS - N_CAR        # 224 pad rows (all on core 7)
K = 6                      # rule-edge slots per car (min-fold handles overflow)
PF = 512 * 4096 // 128     # 16384 param elems per partition per core per param
PTOT = 2 * PF              # 32768: both params

# ---- engine split of the 32768 param elems per partition ----
# Column layout of pq: [ ACT | DVE | PE_sync | PE_gps ]; one DMA per
# range, each issued by the engine whose consumption frees the buffer
# (or an idle engine), per-queue loads sized to finish together.
P_ACT = 7680
P_DVE = 5376
PE_SYNC = 13184
PE_GPS = PTOT - P_ACT - P_DVE - PE_SYNC            # 6528
P_PE = PE_SYNC + PE_GPS                            # 19712 -> 154 chunks
assert PE_SYNC % 128 == 0 and PE_GPS % 128 == 0

SM_TAB = G * K             # 1056 B of fp8 payload table
SM_SCO = 3 * G * 2         # 1056 f16 scores = 2112 B
SM_W = SM_TAB + SM_SCO     # merged smalls tensor width (uint8 bytes)

LAMBDA_RECON, LAMBDA_RULE, LAMBDA_ATTN, LAMBDA_REG = 1.0, 0.5, 0.3, 1e-4
W_ATTN_GAT, W_ATTN_RULE = 0.5, 0.5

_PAD_MS = 0.5              # pad rows: ms=rs=0.5 -> bce adds exactly ln(0.5)

PREP_VER = 7               # bump when prep_in_maps output changes

_NC = None


def _build_nc(repeat=1, sections=("params", "smalls")):
    """Build + compile the per-core Bass program (SPMD; only the input
    shards differ). `repeat` unrolls the body for amortized timing;
    `sections` disables parts for perf bisection (output garbage when
    not all enabled)."""
    do_par = "params" in sections
    do_sm = "smalls" in sections
    nc = bacc.Bacc("TRN2", target_bir_lowering=False, debug=False,
                   enable_asserts=False, num_devices=NCORES)

    # one fused input blob: [ params fp8 | tab fp8 | scores f16 ]
    pq = nc.dram_tensor("pq", [128, PTOT + SM_W], FP8, kind="ExternalInput")
    idd = nc.dram_tensor("idd", [128, 128], BF16, kind="ExternalInput")
    # ACT partials: 0=sln2 1=srule 2=sp_act
    outa = nc.dram_tensor("parts_a", [128, 3], F32, kind="ExternalOutput")
    # DVE partials: 0=sc1 1=sc2 2=nv 3=sar 4=scnt 5=sgat 6=sp_dve 7=sp_pe
    outd = nc.dram_tensor("parts_d", [128, 8], F32, kind="ExternalOutput")

    # contiguous param column ranges
    A0, A1 = 0, P_ACT
    D0, D1 = A1, A1 + P_DVE
    S0, S1 = D1, D1 + PE_SYNC
    Q0, Q1 = S1, PTOT

    with ExitStack() as ctx:
        tc = ctx.enter_context(tile.TileContext(nc))
        pp = ctx.enter_context(tc.tile_pool(name="pp", bufs=3))
        sc = ctx.enter_context(tc.tile_pool(name="sc", bufs=3))
        scr = ctx.enter_context(tc.tile_pool(name="scr", bufs=1))
        ps = ctx.enter_context(tc.psum_pool(name="ps", bufs=4))
        const = ctx.enter_context(tc.tile_pool(name="const", bufs=1))

        ident = const.tile([128, 128], BF16, tag="ident")
        nc.sync.dma_start(ident[:], idd.ap())

        for _rep in range(repeat):
            last = _rep == repeat - 1
            # ---------------- DMAs (3 queues, 4 transfers) ----------------
            # Inputs only — output DMAs go on the last rep so input
            # queues never stall behind compute-dependent stores.
            # ONE giant SWDGE transfer for everything: a single ~4.4MB
            # DMA sustains ~377 GB/s vs ~280 for three ~1.5MB queue
            # streams (SDMA engines are shared across queues; bigger
            # transfers amortize better; sub-4KB descriptors from a
            # separate smalls stream drag the round-robin down). gpsimd
            # issues it (idle engine => free sequencer-side buffer wait).
            t_all = pp.tile([128, PTOT + SM_W], FP8, tag="p_all", bufs=4)
            nc.gpsimd.dma_start(t_all[:], pq.ap())
            if do_sm:
                t_tab = t_all[:, PTOT:PTOT + SM_TAB]
                t_sco = t_all[:, PTOT + SM_TAB:PTOT + SM_W].bitcast(F16)
                t_ms = t_sco[:, 0:G]
                t_rs = t_sco[:, G:2 * G]
                t_bet = t_sco[:, 2 * G:3 * G]
            if do_par:
                t_pa = t_all[:, A0:A1]
                t_pd = t_all[:, D0:D1]
                t_syn = t_all[:, S0:S1]
                t_gps = t_all[:, Q0:Q1]

            out_a = sc.tile([128, 3], F32, tag="out_a")
            out_d = sc.tile([128, 8], F32, tag="out_d")

            # ---------------- PE program ----------------
            # 128-col self-products accumulated into one PSUM tile; the
            # sync-queue block first (arrives on its own queue), then
            # the scalar-queue tail, then the gpsimd tail.
            if do_par:
                psum = ps.tile([128, 128], F32, tag="psum")
                blocks = ([(t_syn, i) for i in range(PE_SYNC // 128)]
                          + [(t_gps, i) for i in range(PE_GPS // 128)])
                nch = len(blocks)
                for c, (t, i) in enumerate(blocks):
                    sl = t[:, 128 * i:128 * (i + 1)]
                    nc.tensor.matmul(psum[:], sl, sl,
                                     start=(c == 0), stop=(c == nch - 1))

            # ---------------- ACT program (smalls first, then params;
            # dd2/sq_d depend only on DVE's earliest ops) ----------------
            if do_sm:
                ln1 = sc.tile([128, G], F16, tag="ln1")
                nc.scalar.activation(ln1[:], t_ms, ACTF.Ln)
                ln2 = sc.tile([128, G], F16, tag="ln2")
                nc.scalar.activation(ln2[:], t_ms, ACTF.Ln, scale=-1.0,
                                     bias=1.0, accum_out=out_a[:, 0:1])
                bsq = sc.tile([128, G], F32, tag="bsq")
                nc.scalar.activation(bsq[:], t_bet, ACTF.Square, scale=-1.0,
                                     bias=1.0)

            # ---------------- DVE smalls ----------------
            if do_sm:
                rowmin = sc.tile([128, G], F32, tag="rowmin")
                nc.vector.tensor_reduce(
                    rowmin[:], t_tab.rearrange("p (g k) -> p g k", k=K),
                    mybir.AxisListType.X, ALU.min)
                viol = sc.tile([128, G], F32, tag="viol")
                nc.vector.tensor_scalar(viol[:], t_rs, 0.5, 0.0, ALU.is_gt,
                                        ALU.add, accum_out=out_d[:, 2:3])
                diff = sc.tile([128, G], F16, tag="diff")
                nc.vector.tensor_tensor(diff[:], t_ms, t_rs, ALU.subtract)

                # ACT follow-ups (deps rowmin/diff now ready; before the
                # big ACT param chunk so DVE's tail never waits on it)
                dd2 = sc.tile([128, G], F32, tag="dd2")
                nc.scalar.activation(dd2[:], rowmin[:], ACTF.Square)
                sq_d = sc.tile([128, G], BF16, tag="sq_d")
                nc.scalar.activation(sq_d[:], diff[:], ACTF.Square,
                                     accum_out=out_a[:, 1:2])

                scr_g = sc.tile([128, G], F16, tag="scr_g")
                nc.vector.scalar_tensor_tensor(scr_g[:], t_rs, 1.0, ln1[:],
                                               ALU.mult, ALU.mult,
                                               accum_out=out_d[:, 0:1])
                scr_g2 = sc.tile([128, G], F16, tag="scr_g2")
                nc.vector.scalar_tensor_tensor(scr_g2[:], t_rs, 1.0, ln2[:],
                                               ALU.mult, ALU.mult,
                                               accum_out=out_d[:, 1:2])
                scr_g3 = sc.tile([128, G], F32, tag="scr_g3")
                nc.vector.scalar_tensor_tensor(scr_g3[:], viol[:], 1.0, bsq[:],
                                               ALU.mult, ALU.mult,
                                               accum_out=out_d[:, 3:4])
                # valid = (rowmin <= 1, i.e. car has a rule edge) * viol
                valid = sc.tile([128, G], F32, tag="valid")
                nc.vector.scalar_tensor_tensor(valid[:], rowmin[:], 1.0,
                                               viol[:], ALU.is_le, ALU.mult,
                                               accum_out=out_d[:, 4:5])
                scr_g4 = sc.tile([128, G], F32, tag="scr_g4")
                nc.vector.scalar_tensor_tensor(scr_g4[:], valid[:], 1.0,
                                               dd2[:], ALU.mult, ALU.mult,
                                               accum_out=out_d[:, 5:6])

            # ---------------- param chunks (tails) ----------------
            if do_par:
                a_scr = scr.tile([128, P_ACT], BF16, tag="a_scr")
                nc.scalar.activation(a_scr[:], t_pa, ACTF.Square,
                                     accum_out=out_a[:, 2:3])
                d_scr = scr.tile([128, P_DVE], BF16, tag="d_scr")
                nc.vector.scalar_tensor_tensor(
                    d_scr[:], t_pd, 1.0, t_pd, ALU.mult, ALU.mult,
                    accum_out=out_d[:, 6:7])
                # diag(PSUM) -> per-partition PE partial
                p_scr = scr.tile([128, 128], F32, tag="p_scr")
                nc.vector.scalar_tensor_tensor(
                    p_scr[:], psum[:], 1.0, ident[:], ALU.mult, ALU.mult,
                    accum_out=out_d[:, 7:8])
            if not do_sm:
                nc.vector.memset(out_a[:, 0:2], 0.0)
                nc.vector.memset(out_d[:, 0:6], 0.0)
            if not do_par:
                nc.vector.memset(out_a[:, 2:3], 0.0)
                nc.vector.memset(out_d[:, 6:8], 0.0)

            if last:
                nc.sync.dma_start(outa.ap(), out_a[:])
                nc.sync.dma_start(outd.ap(), out_d[:])

    nc.compile()
    return nc


def _get_nc():
    global _NC
    if _NC is None:
        _NC = _build_nc()
    return _NC


def prep_in_maps(inputs):
    """Host-side structure prep + sharding. Returns per-core input dicts."""
    ms = np.asarray(inputs["model_scores"], np.float32)
    rsv = np.asarray(inputs["rule_scores"], np.float32)
    alpha = np.asarray(inputs["alpha_gat"], np.float32)
    beta = np.asarray(inputs["beta_rule"], np.float32)
    ei = np.asarray(inputs["edge_index"])
    et = np.asarray(inputs["entity_types"])
    p0 = np.ascontiguousarray(np.asarray(inputs["param0"], np.float32))
    p1 = np.ascontiguousarray(np.asarray(inputs["param1"], np.float32))

    src = ei[0].astype(np.int64, copy=False)
    dst = ei[1].astype(np.int64, copy=False)

    # rule edges: dst is a light (1) or stop line (2)
    rule_node = (et == 1) | (et == 2)
    sel = rule_node[dst]
    src_r = src[sel]
    a_r = alpha[sel]

    # group rule-edge payloads (1 - alpha) by source node (CSR-style)
    order = np.argsort(src_r, kind="stable")
    ssrc = src_r[order]
    pay = np.float32(1.0) - a_r[order]
    counts = np.bincount(ssrc, minlength=N)
    starts = np.zeros_like(counts)
    starts[1:] = np.cumsum(counts[:-1])

    # car ordinal -> node id (reference: nonzero(et==0, size=N_CAR), fill 0)
    car_ids = np.nonzero(et == 0)[0]
    if car_ids.size >= N_CAR:
        car_ids = car_ids[:N_CAR]
    else:
        car_ids = np.concatenate(
            [car_ids, np.zeros(N_CAR - car_ids.size, car_ids.dtype)])

    # [ROWS, K] table of payloads; empty slots = 2.0 (> any payload)
    cnt_full = counts[car_ids]
    cnt_ord = np.minimum(cnt_full, K)
    tot = int(cnt_ord.sum())
    row_idx = np.repeat(np.arange(N_CAR, dtype=np.int64), cnt_ord)
    cum = np.cumsum(cnt_ord) - cnt_ord
    within = np.arange(tot, dtype=np.int64) - np.repeat(cum, cnt_ord)
    srcpos = np.repeat(starts[car_ids], cnt_ord) + within
    ptab = np.full(ROWS * K, 2.0, np.float16)
    ptab[row_idx * K + within] = pay[srcpos]
    # overflow fold (degree > K): min of the extras into the last slot
    over = np.nonzero(cnt_full > K)[0]
    if over.size:
        st = (starts[car_ids[over]] + K).astype(np.int64)
        en = (starts[car_ids[over]] + cnt_full[over]).astype(np.int64)
        bounds = np.stack([st, en], axis=1).ravel()
        mins = np.minimum.reduceat(pay, bounds)[::2]
        idx = over * K + K - 1
        ptab[idx] = np.minimum(ptab[idx], mins.astype(np.float16))
    ptab = ptab.reshape(ROWS, K).astype(ml_dtypes.float8_e4m3)

    # padded score rows
    def pad(v, fill):
        o = np.full(ROWS, fill, np.float32)
        o[:N_CAR] = v
        return o

    # fp16 shipping: ms near 1 would round to exactly 1.0 and make
    # Ln(1-ms) = -inf; clamp to the largest fp16 strictly below 1.
    ms_p = pad(np.minimum(ms, np.float32(1.0 - 2.0 ** -11)), _PAD_MS)
    rs_p = pad(rsv, _PAD_MS)   # pad: bce term exactly ln(0.5); never a viol
    bet_p = pad(beta, 1.0)

    # both params, fp8: [1024, 4096] rows per core -> [128, 32768]
    pq = np.concatenate([p0.reshape(NCORES, 512 * 4096 // PF, PF),
                         p1.reshape(NCORES, 512 * 4096 // PF, PF)],
                        axis=1).astype(ml_dtypes.float8_e4m3)
    # shape now [NCORES, 256, 16384]: per core [128(+128), 16384] halves
    # -> rearrange to [128, 32768] with p0 in cols :16384, p1 in 16384:
    pq = pq.reshape(NCORES, 2, 128, PF).transpose(0, 2, 1, 3).reshape(
        NCORES, 128, PTOT)

    idd = np.eye(128, dtype=ml_dtypes.bfloat16)

    in_maps = []
    for c in range(NCORES):
        r0, r1 = c * RPC, (c + 1) * RPC
        scov = np.concatenate([ms_p[r0:r1].reshape(128, G),
                               rs_p[r0:r1].reshape(128, G),
                               bet_p[r0:r1].reshape(128, G)],
                              axis=1).astype(np.float16)
        tabv = np.ascontiguousarray(ptab[r0:r1]).reshape(128, SM_TAB)
        blob = np.concatenate(
            [pq[c].view(np.uint8), tabv.view(np.uint8),
             scov.view(np.uint8)], axis=1).view(ml_dtypes.float8_e4m3)
        in_maps.append({
            "pq": np.ascontiguousarray(blob),
            "idd": idd,
        })
    return in_maps


def combine_partials(results):
    """Host unshard: sum partial tiles over partitions+cores (f64), apply
    the final scalar formula."""
    sa = np.zeros(3, np.float64)
    sd = np.zeros(8, np.float64)
    for r in results:
        sa += np.asarray(r["parts_a"], np.float64).reshape(128, 3).sum(axis=0)
        sd += np.asarray(r["parts_d"], np.float64).reshape(128, 8).sum(axis=0)
    sln2, srule, sp_act = sa
    sc1, sc2, nv, sar, scnt, sgat, sp_dve, sp_pe = sd
    sp = sp_act + sp_dve + sp_pe
    bce_sum = sc1 + sln2 - sc2
    bce_sum -= NPAD * np.log(0.5)  # remove the constant pad-row contribution

    L_recon = -bce_sum / N_CAR
    L_rule = srule / N_CAR
    any_viol = nv > 0
    L_attn_gat = (sgat / max(scnt, 1.0)) if (any_viol and scnt > 0) else 0.0
    L_attn_rule = (sar / max(nv, 1.0)) if any_viol else 0.0
    L_attn = W_ATTN_GAT * L_attn_gat + W_ATTN_RULE * L_attn_rule
    L_reg = sp
    L_total = (LAMBDA_RECON * L_recon + LAMBDA_RULE * L_rule
               + LAMBDA_ATTN * L_attn + LAMBDA_REG * L_reg)
    return np.array([L_total, L_recon, L_rule, L_attn, L_attn_gat,
                     L_attn_rule, L_reg, nv], np.float32)


def kernel(**inputs):
    nc = _get_nc()
    in_maps = prep_in_maps(inputs)
    res = run_bass_kernel_spmd(nc, in_maps, list(range(NCORES)))
    return combine_partials(res.results)


# revision 13
# speedup vs baseline: 1.0177x; 1.0177x over previous
"""Trainium2 Bass kernel for the ConstraintLoss problem (8-core SPMD).

Contract: kernel(**inputs) takes the FULL unsharded inputs (numpy or jax
arrays, keyed as in setup_inputs()) and returns the full output — the
8-tuple of scalar losses stacked into a float32 array of shape (8,):
  [L_total, L_recon, L_rule, L_attn, L_attn_gat, L_attn_rule, L_reg,
   num_violations]

Sharding strategy (host side = structure prep + shard/unshard only; the
floating-point reduction math runs on the 8 NeuronCores):
  * Cars (180000) are sharded by ordinal range across the 8 cores
    (22528 rows/core, padded); score vectors follow the same row split.
  * The edge-wise segment-max over source-node segments is turned into a
    dense per-car reduction: the host bins each car's rule-edge payloads
    (payload = 1 - alpha, fp8; empty slot = 2.0) into a [rows, K=6]
    table; each core row-MIN-reduces its shard on the vector engine —
    the distributed segment-max from the sharding hint with the node
    space sharded so no cross-core combine is needed. Cars with more
    than K rule edges get the min of the extras folded into the last
    slot on the host (exact).
  * param0/param1 are shipped as fp8 (e4m3) and sharded by rows (512
    rows of each per core); each core computes its partial sum of
    squares split across THREE engines: ACT (Square+accum), DVE (STT
    square+accum), and PE — the tensor engine computes 128-col
    self-products C_k^T C_k accumulated into one PSUM tile; its
    diagonal (per-column sums of squares) is extracted once per
    iteration with an STT against a host-shipped identity matrix.
    fp8 quantization biases L_reg by ~1e-3 relative — far inside the
    2e-2 gate.
  * Each engine accumulates its scalar partials into its OWN [128, C]
    tile (single writer; cross-engine shared accum tiles race on HW)
    and each tile is DMA'd out directly; the host adds partitions and
    cores in float64 and applies the final scalar formula (the
    "all-reduce the scalar losses" step).

DMA: ~4.5 MB per core per iteration as FOUR transfers — one contiguous
column-range of the param blob per queue (sync/scalar HWDGE, gpsimd
SWDGE) plus one merged smalls tensor (tab fp8 + scores f16, bitcast
views on SBUF) — sized so the three queues finish together; the
per-core SDMA aggregate (~320-420 GB/s) is the binding resource.
"""

import numpy as np
from contextlib import ExitStack

import ml_dtypes
import concourse.bacc as bacc
import concourse.mybir as mybir
import concourse.tile as tile
from concourse.bass_utils import run_bass_kernel_spmd

F32 = mybir.dt.float32
F16 = mybir.dt.float16
BF16 = mybir.dt.bfloat16
FP8 = mybir.dt.float8e4
U8 = mybir.dt.uint8
ALU = mybir.AluOpType
ACTF = mybir.ActivationFunctionType

# Problem constants (hardcoded per the task contract).
N_CAR = 180000
N = 200000
NCORES = 8

G = 176                    # row groups per partition
RPC = 128 * G              # 22528 rows (car ordinals) per core
ROWS = RPC * NCORES        # 180224 padded rows
NPAD = ROWS - N_CAR        # 224 pad rows (all on core 7)
K = 6                      # rule-edge slots per car (min-fold handles overflow)
PF = 512 * 4096 // 128     # 16384 param elems per partition per core per param
PTOT = 2 * PF              # 32768: both params

# ---- engine split of the 32768 param elems per partition ----
# Column layout of pq: [ ACT | DVE | PE_sync | PE_gps ]; one DMA per
# range, each issued by the engine whose consumption frees the buffer
# (or an idle engine), per-queue loads sized to finish together.
P_ACT = 7680
P_DVE = 5376
PE_SYNC = 13184
PE_GPS = PTOT - P_ACT - P_DVE - PE_SYNC            # 6528
P_PE = PE_SYNC + PE_GPS                            # 19712 -> 154 chunks
assert PE_SYNC % 128 == 0 and PE_GPS % 128 == 0

SM_TAB = G * K             # 1056 B of fp8 payload table
SM_SCO = 3 * G * 2         # 1056 f16 scores = 2112 B
SM_W = SM_TAB + SM_SCO     # merged smalls tensor width (uint8 bytes)

LAMBDA_RECON, LAMBDA_RULE, LAMBDA_ATTN, LAMBDA_REG = 1.0, 0.5, 0.3, 1e-4
W_ATTN_GAT, W_ATTN_RULE = 0.5, 0.5

_PAD_MS = 0.5              # pad rows: ms=rs=0.5 -> bce adds exactly ln(0.5)

PREP_VER = 7               # bump when prep_in_maps output changes

_NC = None


def _build_nc(repeat=1, sections=("params", "smalls")):
    """Build + compile the per-core Bass program (SPMD; only the input
    shards differ). `repeat` unrolls the body for amortized timing;
    `sections` disables parts for perf bisection (output garbage when
    not all enabled)."""
    do_par = "params" in sections
    do_sm = "smalls" in sections
    nc = bacc.Bacc("TRN2", target_bir_lowering=False, debug=False,
                   enable_asserts=False, num_devices=NCORES)

    # one fused input blob: [ params fp8 | tab fp8 | scores f16 ]
    pq = nc.dram_tensor("pq", [128, PTOT + SM_W], FP8, kind="ExternalInput")
    idd = nc.dram_tensor("idd", [128, 128], BF16, kind="ExternalInput")
    # ACT partials: 0=sln2 1=srule 2=sp_act
    outa = nc.dram_tensor("parts_a", [128, 3], F32, kind="ExternalOutput")
    # DVE partials: 0=sc1 1=sc2 2=nv 3=sar 4=scnt 5=sgat 6=sp_dve 7=sp_pe
    outd = nc.dram_tensor("parts_d", [128, 8], F32, kind="ExternalOutput")

    # contiguous param column ranges
    A0, A1 = 0, P_ACT
    D0, D1 = A1, A1 + P_DVE
    S0, S1 = D1, D1 + PE_SYNC
    Q0, Q1 = S1, PTOT

    with ExitStack() as ctx:
        tc = ctx.enter_context(tile.TileContext(nc))
        pp = ctx.enter_context(tc.tile_pool(name="pp", bufs=3))
        sc = ctx.enter_context(tc.tile_pool(name="sc", bufs=3))
        scr = ctx.enter_context(tc.tile_pool(name="scr", bufs=1))
        ps = ctx.enter_context(tc.psum_pool(name="ps", bufs=4))
        const = ctx.enter_context(tc.tile_pool(name="const", bufs=1))

        ident = const.tile([128, 128], BF16, tag="ident")
        nc.sync.dma_start(ident[:], idd.ap())

        for _rep in range(repeat):
            last = _rep == repeat - 1
            # ---------------- DMAs (3 queues, 4 transfers) ----------------
            # Inputs only — output DMAs go on the last rep so input
            # queues never stall behind compute-dependent stores.
            # ONE giant SWDGE transfer for everything: a single ~4.4MB
            # DMA sustains ~377 GB/s vs ~280 for three ~1.5MB queue
            # streams (SDMA engines are shared across queues; bigger
            # transfers amortize better; sub-4KB descriptors from a
            # separate smalls stream drag the round-robin down). gpsimd
            # issues it (idle engine => free sequencer-side buffer wait).
            t_all = pp.tile([128, PTOT + SM_W], FP8, tag="p_all", bufs=3)
            nc.gpsimd.dma_start(t_all[:], pq.ap())
            if do_sm:
                t_tab = t_all[:, PTOT:PTOT + SM_TAB]
                t_sco = t_all[:, PTOT + SM_TAB:PTOT + SM_W].bitcast(F16)
                t_ms = t_sco[:, 0:G]
                t_rs = t_sco[:, G:2 * G]
                t_bet = t_sco[:, 2 * G:3 * G]
            if do_par:
                t_pa = t_all[:, A0:A1]
                t_pd = t_all[:, D0:D1]
                t_syn = t_all[:, S0:S1]
                t_gps = t_all[:, Q0:Q1]

            out_a = sc.tile([128, 3], F32, tag="out_a")
            out_d = sc.tile([128, 8], F32, tag="out_d")

            # ---------------- PE program ----------------
            # 128-col self-products accumulated into one PSUM tile; the
            # sync-queue block first (arrives on its own queue), then
            # the scalar-queue tail, then the gpsimd tail.
            if do_par:
                psum = ps.tile([128, 128], F32, tag="psum")
                blocks = ([(t_syn, i) for i in range(PE_SYNC // 128)]
                          + [(t_gps, i) for i in range(PE_GPS // 128)])
                nch = len(blocks)
                for c, (t, i) in enumerate(blocks):
                    sl = t[:, 128 * i:128 * (i + 1)]
                    nc.tensor.matmul(psum[:], sl, sl,
                                     start=(c == 0), stop=(c == nch - 1))

            # ---------------- ACT program (smalls first, then params;
            # dd2/sq_d depend only on DVE's earliest ops) ----------------
            if do_sm:
                ln1 = sc.tile([128, G], F16, tag="ln1")
                nc.scalar.activation(ln1[:], t_ms, ACTF.Ln)
                ln2 = sc.tile([128, G], F16, tag="ln2")
                nc.scalar.activation(ln2[:], t_ms, ACTF.Ln, scale=-1.0,
                                     bias=1.0, accum_out=out_a[:, 0:1])
                bsq = sc.tile([128, G], F32, tag="bsq")
                nc.scalar.activation(bsq[:], t_bet, ACTF.Square, scale=-1.0,
                                     bias=1.0)

            # ---------------- DVE smalls ----------------
            if do_sm:
                rowmin = sc.tile([128, G], F32, tag="rowmin")
                nc.vector.tensor_reduce(
                    rowmin[:], t_tab.rearrange("p (g k) -> p g k", k=K),
                    mybir.AxisListType.X, ALU.min)
                viol = sc.tile([128, G], F32, tag="viol")
                nc.vector.tensor_scalar(viol[:], t_rs, 0.5, 0.0, ALU.is_gt,
                                        ALU.add, accum_out=out_d[:, 2:3])
                diff = sc.tile([128, G], F16, tag="diff")
                nc.vector.tensor_tensor(diff[:], t_ms, t_rs, ALU.subtract)

                # ACT follow-ups (deps rowmin/diff now ready; before the
                # big ACT param chunk so DVE's tail never waits on it)
                dd2 = sc.tile([128, G], F32, tag="dd2")
                nc.scalar.activation(dd2[:], rowmin[:], ACTF.Square)
                sq_d = sc.tile([128, G], BF16, tag="sq_d")
                nc.scalar.activation(sq_d[:], diff[:], ACTF.Square,
                                     accum_out=out_a[:, 1:2])

                scr_g = sc.tile([128, G], F16, tag="scr_g")
                nc.vector.scalar_tensor_tensor(scr_g[:], t_rs, 1.0, ln1[:],
                                               ALU.mult, ALU.mult,
                                               accum_out=out_d[:, 0:1])
                scr_g2 = sc.tile([128, G], F16, tag="scr_g2")
                nc.vector.scalar_tensor_tensor(scr_g2[:], t_rs, 1.0, ln2[:],
                                               ALU.mult, ALU.mult,
                                               accum_out=out_d[:, 1:2])
                scr_g3 = sc.tile([128, G], F32, tag="scr_g3")
                nc.vector.scalar_tensor_tensor(scr_g3[:], viol[:], 1.0, bsq[:],
                                               ALU.mult, ALU.mult,
                                               accum_out=out_d[:, 3:4])
                # valid = (rowmin <= 1, i.e. car has a rule edge) * viol
                valid = sc.tile([128, G], F32, tag="valid")
                nc.vector.scalar_tensor_tensor(valid[:], rowmin[:], 1.0,
                                               viol[:], ALU.is_le, ALU.mult,
                                               accum_out=out_d[:, 4:5])
                scr_g4 = sc.tile([128, G], F32, tag="scr_g4")
                nc.vector.scalar_tensor_tensor(scr_g4[:], valid[:], 1.0,
                                               dd2[:], ALU.mult, ALU.mult,
                                               accum_out=out_d[:, 5:6])

            # ---------------- param chunks (tails) ----------------
            if do_par:
                a_scr = scr.tile([128, P_ACT], BF16, tag="a_scr")
                nc.scalar.activation(a_scr[:], t_pa, ACTF.Square,
                                     accum_out=out_a[:, 2:3])
                d_scr = scr.tile([128, P_DVE], BF16, tag="d_scr")
                nc.vector.scalar_tensor_tensor(
                    d_scr[:], t_pd, 1.0, t_pd, ALU.mult, ALU.mult,
                    accum_out=out_d[:, 6:7])
                # diag(PSUM) -> per-partition PE partial
                p_scr = scr.tile([128, 128], F32, tag="p_scr")
                nc.vector.scalar_tensor_tensor(
                    p_scr[:], psum[:], 1.0, ident[:], ALU.mult, ALU.mult,
                    accum_out=out_d[:, 7:8])
            if not do_sm:
                nc.vector.memset(out_a[:, 0:2], 0.0)
                nc.vector.memset(out_d[:, 0:6], 0.0)
            if not do_par:
                nc.vector.memset(out_a[:, 2:3], 0.0)
                nc.vector.memset(out_d[:, 6:8], 0.0)

            if last:
                nc.sync.dma_start(outa.ap(), out_a[:])
                nc.sync.dma_start(outd.ap(), out_d[:])

    nc.compile()
    return nc


def _get_nc():
    global _NC
    if _NC is None:
        _NC = _build_nc()
    return _NC


def prep_in_maps(inputs):
    """Host-side structure prep + sharding. Returns per-core input dicts."""
    ms = np.asarray(inputs["model_scores"], np.float32)
    rsv = np.asarray(inputs["rule_scores"], np.float32)
    alpha = np.asarray(inputs["alpha_gat"], np.float32)
    beta = np.asarray(inputs["beta_rule"], np.float32)
    ei = np.asarray(inputs["edge_index"])
    et = np.asarray(inputs["entity_types"])
    p0 = np.ascontiguousarray(np.asarray(inputs["param0"], np.float32))
    p1 = np.ascontiguousarray(np.asarray(inputs["param1"], np.float32))

    src = ei[0].astype(np.int64, copy=False)
    dst = ei[1].astype(np.int64, copy=False)

    # rule edges: dst is a light (1) or stop line (2)
    rule_node = (et == 1) | (et == 2)
    sel = rule_node[dst]
    src_r = src[sel]
    a_r = alpha[sel]

    # group rule-edge payloads (1 - alpha) by source node (CSR-style)
    order = np.argsort(src_r, kind="stable")
    ssrc = src_r[order]
    pay = np.float32(1.0) - a_r[order]
    counts = np.bincount(ssrc, minlength=N)
    starts = np.zeros_like(counts)
    starts[1:] = np.cumsum(counts[:-1])

    # car ordinal -> node id (reference: nonzero(et==0, size=N_CAR), fill 0)
    car_ids = np.nonzero(et == 0)[0]
    if car_ids.size >= N_CAR:
        car_ids = car_ids[:N_CAR]
    else:
        car_ids = np.concatenate(
            [car_ids, np.zeros(N_CAR - car_ids.size, car_ids.dtype)])

    # [ROWS, K] table of payloads; empty slots = 2.0 (> any payload)
    cnt_full = counts[car_ids]
    cnt_ord = np.minimum(cnt_full, K)
    tot = int(cnt_ord.sum())
    row_idx = np.repeat(np.arange(N_CAR, dtype=np.int64), cnt_ord)
    cum = np.cumsum(cnt_ord) - cnt_ord
    within = np.arange(tot, dtype=np.int64) - np.repeat(cum, cnt_ord)
    srcpos = np.repeat(starts[car_ids], cnt_ord) + within
    ptab = np.full(ROWS * K, 2.0, np.float16)
    ptab[row_idx * K + within] = pay[srcpos]
    # overflow fold (degree > K): min of the extras into the last slot
    over = np.nonzero(cnt_full > K)[0]
    if over.size:
        st = (starts[car_ids[over]] + K).astype(np.int64)
        en = (starts[car_ids[over]] + cnt_full[over]).astype(np.int64)
        bounds = np.stack([st, en], axis=1).ravel()
        mins = np.minimum.reduceat(pay, bounds)[::2]
        idx = over * K + K - 1
        ptab[idx] = np.minimum(ptab[idx], mins.astype(np.float16))
    ptab = ptab.reshape(ROWS, K).astype(ml_dtypes.float8_e4m3)

    # padded score rows
    def pad(v, fill):
        o = np.full(ROWS, fill, np.float32)
        o[:N_CAR] = v
        return o

    # fp16 shipping: ms near 1 would round to exactly 1.0 and make
    # Ln(1-ms) = -inf; clamp to the largest fp16 strictly below 1.
    ms_p = pad(np.minimum(ms, np.float32(1.0 - 2.0 ** -11)), _PAD_MS)
    rs_p = pad(rsv, _PAD_MS)   # pad: bce term exactly ln(0.5); never a viol
    bet_p = pad(beta, 1.0)

    # both params, fp8: [1024, 4096] rows per core -> [128, 32768]
    pq = np.concatenate([p0.reshape(NCORES, 512 * 4096 // PF, PF),
                         p1.reshape(NCORES, 512 * 4096 // PF, PF)],
                        axis=1).astype(ml_dtypes.float8_e4m3)
    # shape now [NCORES, 256, 16384]: per core [128(+128), 16384] halves
    # -> rearrange to [128, 32768] with p0 in cols :16384, p1 in 16384:
    pq = pq.reshape(NCORES, 2, 128, PF).transpose(0, 2, 1, 3).reshape(
        NCORES, 128, PTOT)

    idd = np.eye(128, dtype=ml_dtypes.bfloat16)

    in_maps = []
    for c in range(NCORES):
        r0, r1 = c * RPC, (c + 1) * RPC
        scov = np.concatenate([ms_p[r0:r1].reshape(128, G),
                               rs_p[r0:r1].reshape(128, G),
                               bet_p[r0:r1].reshape(128, G)],
                              axis=1).astype(np.float16)
        tabv = np.ascontiguousarray(ptab[r0:r1]).reshape(128, SM_TAB)
        blob = np.concatenate(
            [pq[c].view(np.uint8), tabv.view(np.uint8),
             scov.view(np.uint8)], axis=1).view(ml_dtypes.float8_e4m3)
        in_maps.append({
            "pq": np.ascontiguousarray(blob),
            "idd": idd,
        })
    return in_maps


def combine_partials(results):
    """Host unshard: sum partial tiles over partitions+cores (f64), apply
    the final scalar formula."""
    sa = np.zeros(3, np.float64)
    sd = np.zeros(8, np.float64)
    for r in results:
        sa += np.asarray(r["parts_a"], np.float64).reshape(128, 3).sum(axis=0)
        sd += np.asarray(r["parts_d"], np.float64).reshape(128, 8).sum(axis=0)
    sln2, srule, sp_act = sa
    sc1, sc2, nv, sar, scnt, sgat, sp_dve, sp_pe = sd
    sp = sp_act + sp_dve + sp_pe
    bce_sum = sc1 + sln2 - sc2
    bce_sum -= NPAD * np.log(0.5)  # remove the constant pad-row contribution

    L_recon = -bce_sum / N_CAR
    L_rule = srule / N_CAR
    any_viol = nv > 0
    L_attn_gat = (sgat / max(scnt, 1.0)) if (any_viol and scnt > 0) else 0.0
    L_attn_rule = (sar / max(nv, 1.0)) if any_viol else 0.0
    L_attn = W_ATTN_GAT * L_attn_gat + W_ATTN_RULE * L_attn_rule
    L_reg = sp
    L_total = (LAMBDA_RECON * L_recon + LAMBDA_RULE * L_rule
               + LAMBDA_ATTN * L_attn + LAMBDA_REG * L_reg)
    return np.array([L_total, L_recon, L_rule, L_attn, L_attn_gat,
                     L_attn_rule, L_reg, nv], np.float32)


def kernel(**inputs):
    nc = _get_nc()
    in_maps = prep_in_maps(inputs)
    res = run_bass_kernel_spmd(nc, in_maps, list(range(NCORES)))
    return combine_partials(res.results)
